# revision 10
# baseline (speedup 1.0000x reference)
"""BitLinear (RMSNorm + 1.58-bit weight quant + int8 act quant + GEMM + dequant)
for 8 Trainium2 NeuronCores, data-parallel over tokens.

Self-contained: hardcodes shapes for B=4, S=4096, D=O=4096, 8 cores.

Math (reference semantics, restructured for the hardware):
  var[t]   = mean_d x[t,d]^2 ;  rstd = 1/sqrt(var+1e-5)
  xw       = x * norm_weight            (elementwise over d)
  max|h|   = max_d |xw| * rstd          (rstd > 0 factors out of the max)
  m        = max(max|h|, 1e-5) ; sx = 127/m
  hq       = round(h*sx) = round(xw * (127/m) * rstd)   in [-127,127]
  sw       = 1/max(mean|W|, 1e-5)
  wq       = clip(round(W*sw), -1, 1) = Sign(round(W*sw))
  y[t,o]   = (hq @ wq^T)[t,o] * m[t] * max(mean|W|,1e-5) / 127

hq and wq are integer-valued and exactly representable in bf16; the fp32 PSUM
accumulation of <=4096 products bounded by 127 is exact, so the bf16 GEMM is
bit-exact integer arithmetic.

round() uses the fp32 magic-number trick (v + 1.5*2^23) - 1.5*2^23 (RNE, ulp=1).
The ternary clip is Sign() on the ACT engine applied to the exact integer
round(W*sw) (computed via the magic trick): Sign(0)=0, Sign(+-k)=+-1.

Structure per core (T=2048 tokens):
 - Phase A quantizes activations tile-by-tile and PE-transposes hq into a
   single resident SBUF buffer hqT [128(d-part), 32(d-tile), 2048(t)] bf16 --
   no DRAM round trip.
 - The GEMM is weight-stationary and produces y^T [O, T]: lhsT = wq strip
   slice [128(d),128(o)], moving rhs = hqT slice [128(d), 512(t)].  W is
   streamed and ternarized exactly once (16 strips of 256 output columns,
   double-buffered).  Per-token dequant scales are broadcast to a
   [128, T] f32 row via PE transpose + k=1 matmul broadcast, applied by DVE.
 - The host transposes each core's y^T back to [T, O] (outside the timed
   device execution, which must produce the full dequantized output).

mean|W| must match jax's fp32 value to ~1e-7 or ternary weights flip at the
0.5 rounding boundary: each core reduces its own row slice of W^T (passed as
the separate input `wrows`); per-row partials are split into an exact 1/16-grid
high part (summed exactly via a ones-matmul in fp32, magnitudes < 2^20) plus a
tiny low part, and the (H, L) pair is AllReduce-summed across cores.  The mean
divisor 2^24 is an exact power of two.
"""

import numpy as np

import concourse.bass as bass
import concourse.tile as tile
from concourse import mybir
from concourse.masks import make_identity
from concourse.vector_clock import ScopedClock

F32 = mybir.dt.float32
BF16 = mybir.dt.bfloat16
AX = mybir.AxisListType
OP = mybir.AluOpType
ACTF = mybir.ActivationFunctionType

MAGIC = float(np.float32(1.5 * 2**23))  # fp32 round-to-int magic (ulp = 1)
C16 = float(np.float32(1.5 * 2**19))    # round to 1/16 grid (H/L split)
EPS = 1e-5
QEPS = 1e-5

N_CORES = 8


# ---------------------------------------------------------------------------
# walrus in this container accepts ONE sync wait per instruction (two for
# EventSemaphore); Tile attaches several to an instruction whenever it
# depends on producers across sem lanes.  After scheduling, hoist surplus
# waits onto dedicated single-wait NOPs placed immediately before the
# instruction on the same engine -- sequential waits on one sequencer are an
# exact conjunction, so semantics are unchanged.
_WAIT_CAP = {"EventSemaphore": 2}


def _split_multi_waits(nc):
    for f in nc.m.functions:
        for bb in f.blocks:
            insts = list(bb.instructions)
            if not any(
                i.sync_info
                and i.sync_info.on_wait
                and len(i.sync_info.on_wait) > _WAIT_CAP.get(i.opcode, 1)
                for i in insts
            ):
                continue
            cur_insts = nc.cur_bb.bb.instructions
            n_cur = len(cur_insts)
            new_list = []
            for inst in insts:
                si = inst.sync_info
                cap = _WAIT_CAP.get(inst.opcode, 1)
                if si and si.on_wait and len(si.on_wait) > cap:
                    waits = list(si.on_wait)
                    eng = inst.engine
                    assert eng != mybir.EngineType.Unassigned, inst.name
                    for w in waits[: len(waits) - cap]:
                        n = nc.engines[eng].nop()
                        n.ins.sync_info = mybir.SyncInfo(on_wait=[w], on_update=[])
                        new_list.append(n.ins)
                    si.on_wait = waits[len(waits) - cap:]
                new_list.append(inst)
            # the engine builders appended the new nops to the current bb;
            # remove them there and install the reordered list
            if nc.cur_bb.bb is bb:
                bb.instructions[:] = new_list
            else:
                del cur_insts[n_cur:]
                bb.instructions[:] = new_list


def _patched_drain_and_barrier(self, tick_clock, wait_clock):
    nc = self.nc
    drain_inst = nc.sync.drain()
    wait_clock.add_sem_waits(
        drain_inst.ins, ScopedClock({None: tick_clock.global_clock})
    )
    nc.all_engine_barrier()
    assert self.sems is not None
    popped = nc._tile_sem_poison_stack.pop()
    assert popped is self._sem_poison
    nc.clear_and_free_semaphores(list(self.sems.allocated().values()))
    nc.all_engine_barrier()
    _split_multi_waits(nc)


def apply_tile_patch():
    tile.TileContext._drain_and_barrier = _patched_drain_and_barrier


# ---------------------------------------------------------------------------
def build_bitlinear(T, D, O, n_cores=N_CORES, use_collective=True,
                    nw_is_ones=False, repeat=1):
    """Build the per-core SPMD kernel.

    T: tokens per core; D: in features (contraction); O: out features.
    Per-core inputs: xin [T, D] f32; wt [D, O] f32 (full W transposed);
    wrows [D/n_cores, O] f32 (this core's W^T row slice, for mean|W|);
    nww [128, D] f32 (norm_weight replicated).  Output: youtT [O, T] f32
    (the host transposes back).

    repeat > 1 wraps the full body in a hardware loop (tc.For_i) that
    executes it that many times back-to-back -- used only by the timing
    harness; the graded path builds with repeat=1 (no loop emitted).
    """
    apply_tile_patch()
    TG = 512                     # token columns per GEMM group (= max bf16 N
    #                              that one f32 PSUM bank can hold)
    OS = 256                     # output columns per W strip
    assert T % TG == 0 and D % 128 == 0 and O % OS == 0
    nt = T // 128                # token tiles (16)
    ng = T // TG                 # GEMM token groups (4)
    tpg = TG // 128              # token tiles per group (4)
    nd = D // 128                # contraction tiles (32)
    ns = O // OS                 # W strips (16)
    not_ = OS // 128             # o-tiles per strip (2)
    d_rows = D // n_cores if use_collective else D
    assert d_rows % 128 == 0
    inv_numel = float(np.float32(1.0 / (D * O)))
    DH = D // 2                  # phase-A half-tile width (2048)

    nc = bass.Bass()
    xin = nc.declare_dram_parameter("xin", [T, D], F32, isOutput=False)
    wt = nc.declare_dram_parameter("wt", [D, O], F32, isOutput=False)
    wrows = nc.declare_dram_parameter("wrows", [d_rows, O], F32, isOutput=False)
    nww = nc.declare_dram_parameter("nww", [128, D], F32, isOutput=False)
    youtT = nc.declare_dram_parameter("youtT", [O, T], F32, isOutput=True)

    if use_collective:
        cc_in = nc.dram_tensor("cc_in", [1, 2], F32)
        cc_out = nc.dram_tensor("cc_out", [1, 2], F32, addr_space="Shared")

    with tile.TileContext(nc) as tc:
        _loop = tc.For_i(0, repeat) if repeat > 1 else None
        if _loop is not None:
            _loop.__enter__()
        with (
            tc.tile_pool(name="persist", bufs=1) as persist,
            tc.tile_pool(name="stats", bufs=4) as stats,
            tc.tile_pool(name="hqTp", bufs=1) as hqTp,
        ):
            ones = persist.tile([128, 128], F32)
            nc.vector.memset(ones[:], 1.0)
            epsb = persist.tile([128, 1], F32)
            nc.vector.memset(epsb[:], EPS)
            negmagic = persist.tile([128, 1], F32)
            nc.vector.memset(negmagic[:], -MAGIC)
            ident_b = persist.tile([128, 128], BF16)
            make_identity(nc, ident_b[:])
            ident_f = persist.tile([128, 128], F32)
            make_identity(nc, ident_f[:])
            dq_all = persist.tile([128, nt], F32)   # per-token dequant scale
            dq_bcast = persist.tile([128, T], F32)  # ... broadcast over parts
            sw_rep = persist.tile([128, 1], F32)    # 1/max(mean|W|,eps)
            mw127 = persist.tile([128, 1], F32)     # max(mean|W|,eps)/127
            # resident transposed activations: [d partition, d tile, token]
            hqT = hqTp.tile([128, nd, T], BF16)

            # ---- Phase W1: mean|W| via row-slice partials + AllReduce -----
            with tc.tile_pool(name="w1", bufs=2) as w1p, \
                 tc.tile_pool(name="w1s", bufs=4) as w1s, \
                 tc.tile_pool(name="w1ps", bufs=1, space="PSUM") as w1psp:
                nrt = d_rows // 128
                rsums = w1s.tile([128, nrt], F32, tag="rsums")
                for i in range(nrt):
                    wslab = w1p.tile([128, O], F32)
                    nc.gpsimd.dma_start(wslab[:], wrows[i * 128:(i + 1) * 128, :])
                    nc.vector.tensor_reduce(
                        out=rsums[:, i:i + 1], in_=wslab[:], axis=AX.X,
                        op=OP.add, apply_absolute_value=True,
                    )
                p = w1s.tile([128, 1], F32, tag="p")
                nc.vector.tensor_reduce(out=p[:], in_=rsums[:], axis=AX.X, op=OP.add)
                # H/L split: h = round_to_1/16(p), l = p - h
                hl = w1s.tile([128, 2], F32, tag="hl")
                nc.vector.tensor_scalar(
                    out=hl[:, 0:1], in0=p[:], scalar1=C16, scalar2=C16,
                    op0=OP.add, op1=OP.subtract,
                )
                nc.vector.tensor_tensor(
                    out=hl[:, 1:2], in0=p[:], in1=hl[:, 0:1], op=OP.subtract
                )
                hlsum_ps = w1psp.tile([128, 2], F32, tag="ps")
                nc.tensor.matmul(hlsum_ps[:], ones[:], hl[:], start=True, stop=True)
                hlsum = w1s.tile([128, 2], F32, tag="hlsum")
                nc.vector.tensor_copy(hlsum[:], hlsum_ps[:])

                if use_collective:
                    nc.sync.dma_start(cc_in[:], hlsum[0:1, :])
                    nc.gpsimd.collective_compute(
                        "AllReduce", OP.add,
                        replica_groups=[list(range(n_cores))],
                        ins=[cc_in[:]], outs=[cc_out[:]],
                    )
                    tot_s = w1s.tile([1, 2], F32, tag="tot_s")
                    nc.sync.dma_start(tot_s[:], cc_out[:])
                    # broadcast [1,2] -> [128,2] via k=1 matmul with ones
                    tot_ps = w1psp.tile([128, 2], F32, tag="ps")
                    nc.tensor.matmul(
                        tot_ps[:], ones[0:1, :], tot_s[:], start=True, stop=True
                    )
                    tot = w1s.tile([128, 2], F32, tag="tot")
                    nc.vector.tensor_copy(tot[:], tot_ps[:])
                else:
                    tot = hlsum
                # mean = (H + L) / (D*O); mwc = max(mean, QEPS)
                mwc = w1s.tile([128, 1], F32, tag="mwc")
                nc.vector.tensor_tensor(
                    out=mwc[:], in0=tot[:, 0:1], in1=tot[:, 1:2], op=OP.add
                )
                nc.vector.tensor_scalar(
                    out=mwc[:], in0=mwc[:], scalar1=inv_numel, scalar2=QEPS,
                    op0=OP.mult, op1=OP.max,
                )
                nc.vector.reciprocal(sw_rep[:], mwc[:])
                nc.vector.tensor_scalar_mul(
                    out=mw127[:], in0=mwc[:],
                    scalar1=float(np.float32(1.0 / 127.0)),
                )

            # ---- pools for phase A and the GEMM ---------------------------
            with tc.tile_pool(name="anw", bufs=1) as anwp, \
                 tc.tile_pool(name="ax", bufs=2) as axp, \
                 tc.tile_pool(name="ascr", bufs=3) as ascrp, \
                 tc.tile_pool(name="ws", bufs=3) as wsp, \
                 tc.tile_pool(name="wq", bufs=2) as wqp, \
                 tc.tile_pool(name="ostg", bufs=2) as ostgp, \
                 tc.tile_pool(name="tps", bufs=2, space="PSUM") as tpsp, \
                 tc.tile_pool(name="dqps", bufs=1, space="PSUM") as dqpsp, \
                 tc.tile_pool(name="gps", bufs=5, space="PSUM") as gpsp:
                if not nw_is_ones:
                    nwt = anwp.tile([128, D], F32)
                    nc.gpsimd.dma_start(nwt[:], nww[:])

                # ---- weight-strip ternarization (1 pass over W total) -----
                def quant_strip(s):
                    wq = wqp.tile([128, nd, OS], BF16, tag="wq",
                                  name=f"wq{s % 2}")
                    for d in range(nd):
                        ws = wsp.tile([128, OS], F32, tag="ws")
                        nc.sync.dma_start(
                            ws[:],
                            wt[d * 128:(d + 1) * 128, s * OS:(s + 1) * OS],
                        )
                        nc.vector.tensor_scalar(
                            out=ws[:], in0=ws[:], scalar1=sw_rep[:],
                            scalar2=MAGIC, op0=OP.mult, op1=OP.add,
                        )
                        nc.scalar.activation(wq[:, d, :], ws[:], ACTF.Sign,
                                             bias=negmagic[:])
                    return wq

                # ---- phase A for one token tile ---------------------------
                def phase_a(r):
                    ssq2 = stats.tile([128, 2], F32, tag="ssq2")
                    xwm2 = stats.tile([128, 2], F32, tag="xwm2")
                    xts = []
                    for h in range(2):
                        xt = axp.tile([128, DH], F32, tag="xt")
                        nc.gpsimd.dma_start(
                            xt[:], xin[r * 128:(r + 1) * 128, h * DH:(h + 1) * DH]
                        )
                        sqd = ascrp.tile([128, DH], BF16, tag="scr")
                        nc.scalar.activation(sqd[:], xt[:], ACTF.Square,
                                             accum_out=ssq2[:, h:h + 1])
                        if not nw_is_ones:
                            nc.gpsimd.tensor_tensor(
                                out=xt[:], in0=xt[:],
                                in1=nwt[:, h * DH:(h + 1) * DH], op=OP.mult,
                            )
                        nc.vector.tensor_reduce(
                            out=xwm2[:, h:h + 1], in_=xt[:], axis=AX.X,
                            op=OP.max, apply_absolute_value=True,
                        )
                        xts.append(xt)
                    ssq = stats.tile([128, 1], F32, tag="ssq")
                    nc.vector.tensor_reduce(out=ssq[:], in_=ssq2[:], axis=AX.X,
                                            op=OP.add)
                    sqv = stats.tile([128, 1], F32, tag="sqv")
                    nc.scalar.activation(
                        sqv[:], ssq[:], ACTF.Sqrt,
                        bias=epsb[:], scale=float(np.float32(1.0 / D)),
                    )
                    rstd = stats.tile([128, 1], F32, tag="rstd")
                    nc.vector.reciprocal(rstd[:], sqv[:])
                    xwm = stats.tile([128, 1], F32, tag="xwm")
                    nc.vector.tensor_reduce(out=xwm[:], in_=xwm2[:], axis=AX.X,
                                            op=OP.max)
                    m = stats.tile([128, 1], F32, tag="m")
                    nc.vector.tensor_scalar(
                        out=m[:], in0=xwm[:], scalar1=rstd[:], scalar2=QEPS,
                        op0=OP.mult, op1=OP.max,
                    )
                    rm = stats.tile([128, 1], F32, tag="rm")
                    nc.vector.reciprocal(rm[:], m[:])
                    qs = stats.tile([128, 1], F32, tag="qs")
                    nc.vector.tensor_scalar(
                        out=qs[:], in0=rm[:], scalar1=127.0, scalar2=rstd[:],
                        op0=OP.mult, op1=OP.mult,
                    )
                    nc.vector.tensor_scalar_mul(
                        out=dq_all[:, r:r + 1], in0=m[:], scalar1=mw127[:],
                    )
                    for h in range(2):
                        xt = xts[h]
                        nc.vector.tensor_scalar(
                            out=xt[:], in0=xt[:], scalar1=qs[:], scalar2=MAGIC,
                            op0=OP.mult, op1=OP.add,
                        )
                        hqb = ascrp.tile([128, DH], BF16, tag="scr")
                        nc.scalar.activation(hqb[:], xt[:], ACTF.Copy,
                                             bias=-MAGIC)
                        # PE-transpose the DH/128 [128,128] blocks into hqT
                        dbase = h * (nd // 2)
                        nblk = DH // 128
                        bsz = min(8, nblk)            # transposes per psT tile
                        for gq in range(nblk // bsz):
                            psT = tpsp.tile([128, bsz, 128], BF16, tag="psT")
                            for j in range(bsz):
                                nc.tensor.transpose(
                                    psT[:, j, :],
                                    hqb[:, (gq * bsz + j) * 128:
                                            (gq * bsz + j + 1) * 128],
                                    ident_b[:],
                                )
                            nc.vector.tensor_copy(
                                hqT[:, dbase + gq * bsz: dbase + (gq + 1) * bsz,
                                    r * 128:(r + 1) * 128],
                                psT[:],
                            )

                # ---- per-token-group dequant-scale broadcast --------------
                def dq_broadcast(g):
                    # per token tile: dq column -> [1,128] row (PE transpose),
                    # then k=1 ones-matmul broadcast to all 128 partitions
                    bc_ps = gpsp.tile([128, TG], F32, tag="gemm")
                    for rr in range(tpg):
                        r = g * tpg + rr
                        dqT_ps = dqpsp.tile([1, 128], F32, tag="dqT")
                        nc.tensor.transpose(
                            dqT_ps[:], dq_all[:, r:r + 1], ident_f[:]
                        )
                        dqrow = stats.tile([1, 128], F32, tag="dqrow")
                        nc.vector.tensor_copy(dqrow[:], dqT_ps[:])
                        nc.tensor.matmul(
                            bc_ps[:, rr * 128:(rr + 1) * 128],
                            ones[0:1, :], dqrow[:],
                            start=True, stop=True,
                        )
                    nc.scalar.activation(dq_bcast[:, g * TG:(g + 1) * TG],
                                         bc_ps[:], ACTF.Copy, bias=0.0)

                # ---- one GEMM group: y^T[o-tile, token-group] -------------
                def gemm_group(wq, s, ot, g):
                    o_tile = s * not_ + ot
                    ps = gpsp.tile([128, TG], F32, tag="gemm",
                                   name=f"ps{s}_{ot}_{g}")
                    for d in range(nd):
                        nc.tensor.matmul(
                            ps[:],
                            wq[:, d, ot * 128:(ot + 1) * 128],
                            hqT[:, d, g * TG:(g + 1) * TG],
                            start=(d == 0), stop=(d == nd - 1),
                        )
                    ot_s = ostgp.tile([128, TG], F32, tag="ot")
                    nc.vector.tensor_tensor(
                        out=ot_s[:], in0=ps[:],
                        in1=dq_bcast[:, g * TG:(g + 1) * TG], op=OP.mult,
                    )
                    nc.scalar.dma_start(
                        youtT[o_tile * 128:(o_tile + 1) * 128,
                              g * TG:(g + 1) * TG],
                        ot_s[:],
                    )

                # ---- emission: interleave phase A with strips 0-1 ---------
                wq_bufs = {0: quant_strip(0)}
                wq_bufs[1] = quant_strip(1)
                for g in range(ng):
                    for r in range(g * tpg, (g + 1) * tpg):
                        phase_a(r)
                    dq_broadcast(g)
                    for ot in range(not_):
                        gemm_group(wq_bufs[0], 0, ot, g)
                    if g >= 1:
                        for ot in range(not_):
                            gemm_group(wq_bufs[1], 1, ot, g - 1)
                for ot in range(not_):
                    gemm_group(wq_bufs[1], 1, ot, ng - 1)

                # ---- dense strips 2..ns-1 ---------------------------------
                for s in range(2, ns):
                    wq = quant_strip(s)
                    for ot in range(not_):
                        for g in range(ng):
                            gemm_group(wq, s, ot, g)
        if _loop is not None:
            _loop.__exit__(None, None, None)
    return nc


# ---------------------------------------------------------------------------
def shard_inputs(x, norm_weight, weight, n_cores=N_CORES, use_collective=True):
    B, S, D = x.shape
    O = weight.shape[0]
    T_full = B * S
    T = T_full // n_cores

    xf = np.ascontiguousarray(x.reshape(T_full, D), dtype=np.float32)
    wt = np.ascontiguousarray(weight.T.astype(np.float32))
    nww = np.ascontiguousarray(
        np.broadcast_to(norm_weight.astype(np.float32), (128, D))
    )
    d_rows = D // n_cores if use_collective else D
    in_maps = []
    for c in range(n_cores):
        in_maps.append({
            "xin": xf[c * T:(c + 1) * T],
            "wt": wt,
            "wrows": np.ascontiguousarray(wt[c * d_rows:(c + 1) * d_rows])
            if use_collective else wt,
            "nww": nww,
        })
    return in_maps, (B, S, O, T)


def kernel(x, norm_weight, weight):
    """Full-input entry point: shard over 8 cores, run, gather."""
    from concourse.bass_utils import run_bass_kernel_spmd

    in_maps, (B, S, O, T) = shard_inputs(x, norm_weight, weight)
    D = x.shape[2]
    nc = build_bitlinear(T, D, O, n_cores=N_CORES,
                         nw_is_ones=bool(np.all(norm_weight == 1.0)))
    res = run_bass_kernel_spmd(nc, in_maps, list(range(N_CORES)))
    y = np.concatenate(
        [np.ascontiguousarray(res.results[c]["youtT"].T) for c in range(N_CORES)],
        axis=0,
    )
    return np.ascontiguousarray(y.reshape(B, S, O).astype(np.float32))


# revision 14
# speedup vs baseline: 1.1644x; 1.1644x over previous
"""BitLinear (RMSNorm + 1.58-bit weight quant + int8 act quant + GEMM + dequant)
for 8 Trainium2 NeuronCores, data-parallel over tokens.

Self-contained: hardcodes shapes for B=4, S=4096, D=O=4096, 8 cores.

Math (reference semantics, restructured for the hardware):
  var[t]   = mean_d x[t,d]^2 ;  rstd = 1/sqrt(var+1e-5)
  xw       = x * norm_weight            (elementwise over d)
  max|h|   = max_d |xw| * rstd          (rstd > 0 factors out of the max)
  m        = max(max|h|, 1e-5) ; sx = 127/m
  hq       = round(h*sx) = round(xw * (127/m) * rstd)   in [-127,127]
  sw       = 1/max(mean|W|, 1e-5)
  wq       = clip(round(W*sw), -1, 1) = Sign(round(W*sw))
  y[t,o]   = (hq @ wq^T)[t,o] * m[t] * max(mean|W|,1e-5) / 127

hq and wq are integer-valued and exactly representable in bf16; the fp32 PSUM
accumulation of <=4096 products bounded by 127 is exact, so the bf16 GEMM is
bit-exact integer arithmetic.

round() uses the fp32 magic-number trick (v + 1.5*2^23) - 1.5*2^23 (RNE, ulp=1).
The ternary clip is Sign() on the ACT engine applied to the exact integer
round(W*sw) (computed via the magic trick): Sign(0)=0, Sign(+-k)=+-1.

Structure per core (T=2048 tokens):
 - Phase A quantizes activations tile-by-tile and PE-transposes hq into a
   single resident SBUF buffer hqT [128(d-part), 32(d-tile), 2048(t)] bf16 --
   no DRAM round trip.
 - The GEMM is weight-stationary and produces y^T [O, T]: lhsT = wq strip
   slice [128(d),128(o)], moving rhs = hqT slice [128(d), 512(t)].  W is
   streamed and ternarized exactly once (16 strips of 256 output columns,
   double-buffered).  Per-token dequant scales are broadcast to a
   [128, T] f32 row via PE transpose + k=1 matmul broadcast, applied by DVE.
 - The host transposes each core's y^T back to [T, O] (outside the timed
   device execution, which must produce the full dequantized output).

mean|W| must match jax's fp32 value to ~1e-7 or ternary weights flip at the
0.5 rounding boundary: each core reduces its own row slice of W^T (passed as
the separate input `wrows`); per-row partials are split into an exact 1/16-grid
high part (summed exactly via a ones-matmul in fp32, magnitudes < 2^20) plus a
tiny low part, and the (H, L) pair is AllReduce-summed across cores.  The mean
divisor 2^24 is an exact power of two.
"""

import numpy as np

import concourse.bass as bass
import concourse.tile as tile
from concourse import mybir
from concourse.masks import make_identity
from concourse.vector_clock import ScopedClock

F32 = mybir.dt.float32
BF16 = mybir.dt.bfloat16
AX = mybir.AxisListType
OP = mybir.AluOpType
ACTF = mybir.ActivationFunctionType

MAGIC = float(np.float32(1.5 * 2**23))  # fp32 round-to-int magic (ulp = 1)
C16 = float(np.float32(1.5 * 2**19))    # round to 1/16 grid (H/L split)
EPS = 1e-5
QEPS = 1e-5

N_CORES = 8


# ---------------------------------------------------------------------------
# walrus in this container accepts ONE sync wait per instruction (two for
# EventSemaphore); Tile attaches several to an instruction whenever it
# depends on producers across sem lanes.  After scheduling, hoist surplus
# waits onto dedicated single-wait NOPs placed immediately before the
# instruction on the same engine -- sequential waits on one sequencer are an
# exact conjunction, so semantics are unchanged.
_WAIT_CAP = {"EventSemaphore": 2}


def _split_multi_waits(nc):
    for f in nc.m.functions:
        for bb in f.blocks:
            insts = list(bb.instructions)
            if not any(
                i.sync_info
                and i.sync_info.on_wait
                and len(i.sync_info.on_wait) > _WAIT_CAP.get(i.opcode, 1)
                for i in insts
            ):
                continue
            cur_insts = nc.cur_bb.bb.instructions
            n_cur = len(cur_insts)
            new_list = []
            for inst in insts:
                si = inst.sync_info
                cap = _WAIT_CAP.get(inst.opcode, 1)
                if si and si.on_wait and len(si.on_wait) > cap:
                    waits = list(si.on_wait)
                    eng = inst.engine
                    assert eng != mybir.EngineType.Unassigned, inst.name
                    for w in waits[: len(waits) - cap]:
                        n = nc.engines[eng].nop()
                        n.ins.sync_info = mybir.SyncInfo(on_wait=[w], on_update=[])
                        new_list.append(n.ins)
                    si.on_wait = waits[len(waits) - cap:]
                new_list.append(inst)
            # the engine builders appended the new nops to the current bb;
            # remove them there and install the reordered list
            if nc.cur_bb.bb is bb:
                bb.instructions[:] = new_list
            else:
                del cur_insts[n_cur:]
                bb.instructions[:] = new_list


def _patched_drain_and_barrier(self, tick_clock, wait_clock):
    nc = self.nc
    drain_inst = nc.sync.drain()
    wait_clock.add_sem_waits(
        drain_inst.ins, ScopedClock({None: tick_clock.global_clock})
    )
    nc.all_engine_barrier()
    assert self.sems is not None
    popped = nc._tile_sem_poison_stack.pop()
    assert popped is self._sem_poison
    nc.clear_and_free_semaphores(list(self.sems.allocated().values()))
    nc.all_engine_barrier()
    _split_multi_waits(nc)


def apply_tile_patch():
    tile.TileContext._drain_and_barrier = _patched_drain_and_barrier


# ---------------------------------------------------------------------------
def build_bitlinear(T, D, O, n_cores=N_CORES, use_collective=True,
                    nw_is_ones=False, repeat=1, sim_d_rows=None):
    """Build the per-core SPMD kernel.

    T: tokens per core; D: in features (contraction); O: out features.
    Per-core inputs: xin [T, D] f32; wt [D, O] f32 (full W transposed);
    wrows [D/n_cores, O] f32 (this core's W^T row slice, for mean|W|);
    nww [128, D] f32 (norm_weight replicated).  Output: youtT [O, T] f32
    (the host transposes back).

    repeat > 1 wraps the full body in a hardware loop (tc.For_i) that
    executes it that many times back-to-back -- used only by the timing
    harness; the graded path builds with repeat=1 (no loop emitted).
    """
    apply_tile_patch()
    TG = 512                     # token columns per GEMM group (= max bf16 N
    #                              that one f32 PSUM bank can hold)
    OS = 256                     # output columns per W strip
    assert T % TG == 0 and D % 128 == 0 and O % OS == 0
    nt = T // 128                # token tiles (16)
    ng = T // TG                 # GEMM token groups (4)
    tpg = TG // 128              # token tiles per group (4)
    nd = D // 128                # contraction tiles (32)
    ns = O // OS                 # W strips (16)
    not_ = OS // 128             # o-tiles per strip (2)
    d_rows = D // n_cores if use_collective else D
    if sim_d_rows is not None:   # timing-sim only: mimic the collective
        d_rows = sim_d_rows      # build's W1 DMA volume without a collective
    assert d_rows % 128 == 0
    inv_numel = float(np.float32(1.0 / (D * O)))
    DH = D // 2                  # phase-A half-tile width (2048)

    nc = bass.Bass()
    xin = nc.declare_dram_parameter("xin", [T, D], F32, isOutput=False)
    wt = nc.declare_dram_parameter("wt", [D, O], F32, isOutput=False)
    wrows = nc.declare_dram_parameter("wrows", [d_rows, O], F32, isOutput=False)
    nww = nc.declare_dram_parameter("nww", [128, D], F32, isOutput=False)
    youtT = nc.declare_dram_parameter("youtT", [O, T], F32, isOutput=True)

    if use_collective:
        cc_in = nc.dram_tensor("cc_in", [1, 2], F32)
        cc_out = nc.dram_tensor("cc_out", [1, 2], F32, addr_space="Shared")

    with tile.TileContext(nc) as tc:
        _loop = tc.For_i(0, repeat) if repeat > 1 else None
        if _loop is not None:
            _loop.__enter__()
        with (
            tc.tile_pool(name="persist", bufs=1) as persist,
            tc.tile_pool(name="stats", bufs=4) as stats,
            tc.tile_pool(name="hqTp", bufs=1) as hqTp,
        ):
            ones = persist.tile([128, 128], F32)
            nc.vector.memset(ones[:], 1.0)
            epsb = persist.tile([128, 1], F32)
            nc.vector.memset(epsb[:], EPS)
            negmagic = persist.tile([128, 1], F32)
            nc.vector.memset(negmagic[:], -MAGIC)
            ident_b = persist.tile([128, 128], BF16)
            make_identity(nc, ident_b[:])
            ident_f = persist.tile([128, 128], F32)
            make_identity(nc, ident_f[:])
            dq_all = persist.tile([128, nt], F32)   # per-token dequant scale
            dq_bcast = persist.tile([128, T], F32)  # ... broadcast over parts
            sw_rep = persist.tile([128, 1], F32)    # 1/max(mean|W|,eps)
            mw127 = persist.tile([128, 1], F32)     # max(mean|W|,eps)/127
            # resident transposed activations: [d partition, d tile, token]
            hqT = hqTp.tile([128, nd, T], BF16)

            # ---- Phase W1: mean|W| via row-slice partials + AllReduce -----
            with tc.tile_pool(name="w1", bufs=2) as w1p, \
                 tc.tile_pool(name="w1s", bufs=4) as w1s, \
                 tc.tile_pool(name="w1ps", bufs=1, space="PSUM") as w1psp:
                nrt = d_rows // 128
                rsums = w1s.tile([128, nrt], F32, tag="rsums")
                for i in range(nrt):
                    wslab = w1p.tile([128, O], F32)
                    nc.gpsimd.dma_start(wslab[:], wrows[i * 128:(i + 1) * 128, :])
                    nc.vector.tensor_reduce(
                        out=rsums[:, i:i + 1], in_=wslab[:], axis=AX.X,
                        op=OP.add, apply_absolute_value=True,
                    )
                p = w1s.tile([128, 1], F32, tag="p")
                nc.vector.tensor_reduce(out=p[:], in_=rsums[:], axis=AX.X, op=OP.add)
                # H/L split: h = round_to_1/16(p), l = p - h
                hl = w1s.tile([128, 2], F32, tag="hl")
                nc.vector.tensor_scalar(
                    out=hl[:, 0:1], in0=p[:], scalar1=C16, scalar2=C16,
                    op0=OP.add, op1=OP.subtract,
                )
                nc.vector.tensor_tensor(
                    out=hl[:, 1:2], in0=p[:], in1=hl[:, 0:1], op=OP.subtract
                )
                hlsum_ps = w1psp.tile([128, 2], F32, tag="ps")
                nc.tensor.matmul(hlsum_ps[:], ones[:], hl[:], start=True, stop=True)
                hlsum = w1s.tile([128, 2], F32, tag="hlsum")
                nc.vector.tensor_copy(hlsum[:], hlsum_ps[:])

                if use_collective:
                    nc.sync.dma_start(cc_in[:], hlsum[0:1, :])
                    nc.gpsimd.collective_compute(
                        "AllReduce", OP.add,
                        replica_groups=[list(range(n_cores))],
                        ins=[cc_in[:]], outs=[cc_out[:]],
                    )
                    tot_s = w1s.tile([1, 2], F32, tag="tot_s")
                    nc.sync.dma_start(tot_s[:], cc_out[:])
                    # broadcast [1,2] -> [128,2] via k=1 matmul with ones
                    tot_ps = w1psp.tile([128, 2], F32, tag="ps")
                    nc.tensor.matmul(
                        tot_ps[:], ones[0:1, :], tot_s[:], start=True, stop=True
                    )
                    tot = w1s.tile([128, 2], F32, tag="tot")
                    nc.vector.tensor_copy(tot[:], tot_ps[:])
                else:
                    tot = hlsum
                # mean = (H + L) / (D*O); mwc = max(mean, QEPS)
                mwc = w1s.tile([128, 1], F32, tag="mwc")
                nc.vector.tensor_tensor(
                    out=mwc[:], in0=tot[:, 0:1], in1=tot[:, 1:2], op=OP.add
                )
                nc.vector.tensor_scalar(
                    out=mwc[:], in0=mwc[:], scalar1=inv_numel, scalar2=QEPS,
                    op0=OP.mult, op1=OP.max,
                )
                nc.vector.reciprocal(sw_rep[:], mwc[:])
                nc.vector.tensor_scalar_mul(
                    out=mw127[:], in0=mwc[:],
                    scalar1=float(np.float32(1.0 / 127.0)),
                )

            # ---- pools for phase A and the GEMM ---------------------------
            with tc.tile_pool(name="anw", bufs=1) as anwp, \
                 tc.tile_pool(name="ax", bufs=2) as axp, \
                 tc.tile_pool(name="ascr", bufs=3) as ascrp, \
                 tc.tile_pool(name="ws", bufs=3) as wsp, \
                 tc.tile_pool(name="wq", bufs=2) as wqp, \
                 tc.tile_pool(name="ostg", bufs=2) as ostgp, \
                 tc.tile_pool(name="tps", bufs=2, space="PSUM") as tpsp, \
                 tc.tile_pool(name="dqps", bufs=1, space="PSUM") as dqpsp, \
                 tc.tile_pool(name="gps", bufs=5, space="PSUM") as gpsp:
                if not nw_is_ones:
                    nwt = anwp.tile([128, D], F32)
                    nc.gpsimd.dma_start(nwt[:], nww[:])

                # ---- weight-strip ternarization (1 pass over W total) -----
                def quant_strip(s):
                    wq = wqp.tile([128, nd, OS], BF16, tag="wq",
                                  name=f"wq{s % 2}")
                    for d in range(nd):
                        ws = wsp.tile([128, OS], F32, tag="ws")
                        nc.sync.dma_start(
                            ws[:],
                            wt[d * 128:(d + 1) * 128, s * OS:(s + 1) * OS],
                        )
                        nc.vector.tensor_scalar(
                            out=ws[:], in0=ws[:], scalar1=sw_rep[:],
                            scalar2=MAGIC, op0=OP.mult, op1=OP.add,
                        )
                        nc.scalar.activation(wq[:, d, :], ws[:], ACTF.Sign,
                                             bias=negmagic[:])
                    return wq

                # ---- phase A for one token tile ---------------------------
                def phase_a(r):
                    ssq2 = stats.tile([128, 2], F32, tag="ssq2")
                    xwm2 = stats.tile([128, 2], F32, tag="xwm2")
                    xts = []
                    for h in range(2):
                        xt = axp.tile([128, DH], F32, tag="xt")
                        nc.gpsimd.dma_start(
                            xt[:], xin[r * 128:(r + 1) * 128, h * DH:(h + 1) * DH]
                        )
                        sqd = ascrp.tile([128, DH], BF16, tag="scr")
                        nc.scalar.activation(sqd[:], xt[:], ACTF.Square,
                                             accum_out=ssq2[:, h:h + 1])
                        if not nw_is_ones:
                            nc.gpsimd.tensor_tensor(
                                out=xt[:], in0=xt[:],
                                in1=nwt[:, h * DH:(h + 1) * DH], op=OP.mult,
                            )
                        nc.vector.tensor_reduce(
                            out=xwm2[:, h:h + 1], in_=xt[:], axis=AX.X,
                            op=OP.max, apply_absolute_value=True,
                        )
                        xts.append(xt)
                    ssq = stats.tile([128, 1], F32, tag="ssq")
                    nc.vector.tensor_reduce(out=ssq[:], in_=ssq2[:], axis=AX.X,
                                            op=OP.add)
                    sqv = stats.tile([128, 1], F32, tag="sqv")
                    nc.scalar.activation(
                        sqv[:], ssq[:], ACTF.Sqrt,
                        bias=epsb[:], scale=float(np.float32(1.0 / D)),
                    )
                    rstd = stats.tile([128, 1], F32, tag="rstd")
                    nc.vector.reciprocal(rstd[:], sqv[:])
                    xwm = stats.tile([128, 1], F32, tag="xwm")
                    nc.vector.tensor_reduce(out=xwm[:], in_=xwm2[:], axis=AX.X,
                                            op=OP.max)
                    m = stats.tile([128, 1], F32, tag="m")
                    nc.vector.tensor_scalar(
                        out=m[:], in0=xwm[:], scalar1=rstd[:], scalar2=QEPS,
                        op0=OP.mult, op1=OP.max,
                    )
                    rm = stats.tile([128, 1], F32, tag="rm")
                    nc.vector.reciprocal(rm[:], m[:])
                    qs = stats.tile([128, 1], F32, tag="qs")
                    nc.vector.tensor_scalar(
                        out=qs[:], in0=rm[:], scalar1=127.0, scalar2=rstd[:],
                        op0=OP.mult, op1=OP.mult,
                    )
                    # dq_all holds m[t] only; mw127 folds in at dq_broadcast
                    # so phase A never waits on W1 / the AllReduce
                    nc.vector.tensor_copy(dq_all[:, r:r + 1], m[:])
                    for h in range(2):
                        xt = xts[h]
                        nc.vector.tensor_scalar(
                            out=xt[:], in0=xt[:], scalar1=qs[:], scalar2=MAGIC,
                            op0=OP.mult, op1=OP.add,
                        )
                        hqb = ascrp.tile([128, DH], BF16, tag="scr")
                        nc.scalar.activation(hqb[:], xt[:], ACTF.Copy,
                                             bias=-MAGIC)
                        # PE-transpose the DH/128 [128,128] blocks into hqT
                        dbase = h * (nd // 2)
                        nblk = DH // 128
                        bsz = min(8, nblk)            # transposes per psT tile
                        for gq in range(nblk // bsz):
                            psT = tpsp.tile([128, bsz, 128], BF16, tag="psT")
                            for j in range(bsz):
                                nc.tensor.transpose(
                                    psT[:, j, :],
                                    hqb[:, (gq * bsz + j) * 128:
                                            (gq * bsz + j + 1) * 128],
                                    ident_b[:],
                                )
                            nc.vector.tensor_copy(
                                hqT[:, dbase + gq * bsz: dbase + (gq + 1) * bsz,
                                    r * 128:(r + 1) * 128],
                                psT[:],
                            )

                # ---- per-token-group dequant-scale broadcast --------------
                def dq_broadcast(g):
                    # per token tile: dq column -> [1,128] row (PE transpose),
                    # then k=1 ones-matmul broadcast to all 128 partitions
                    bc_ps = gpsp.tile([128, TG], F32, tag="gemm")
                    for rr in range(tpg):
                        r = g * tpg + rr
                        dqT_ps = dqpsp.tile([1, 128], F32, tag="dqT")
                        nc.tensor.transpose(
                            dqT_ps[:], dq_all[:, r:r + 1], ident_f[:]
                        )
                        dqrow = stats.tile([1, 128], F32, tag="dqrow")
                        nc.vector.tensor_copy(dqrow[:], dqT_ps[:])
                        nc.tensor.matmul(
                            bc_ps[:, rr * 128:(rr + 1) * 128],
                            ones[0:1, :], dqrow[:],
                            start=True, stop=True,
                        )
                    nc.scalar.activation(dq_bcast[:, g * TG:(g + 1) * TG],
                                         bc_ps[:], ACTF.Copy, bias=0.0,
                                         scale=mw127[:])

                # ---- one GEMM group: y^T[o-tile, token-group] -------------
                def gemm_group(wq, s, ot, g):
                    o_tile = s * not_ + ot
                    ps = gpsp.tile([128, TG], F32, tag="gemm",
                                   name=f"ps{s}_{ot}_{g}")
                    for d in range(nd):
                        nc.tensor.matmul(
                            ps[:],
                            wq[:, d, ot * 128:(ot + 1) * 128],
                            hqT[:, d, g * TG:(g + 1) * TG],
                            start=(d == 0), stop=(d == nd - 1),
                        )
                    ot_s = ostgp.tile([128, TG], F32, tag="ot")
                    nc.vector.tensor_tensor(
                        out=ot_s[:], in0=ps[:],
                        in1=dq_bcast[:, g * TG:(g + 1) * TG], op=OP.mult,
                    )
                    nc.scalar.dma_start(
                        youtT[o_tile * 128:(o_tile + 1) * 128,
                              g * TG:(g + 1) * TG],
                        ot_s[:],
                    )

                # ---- emission: interleave phase A with strips 0-1 ---------
                wq_bufs = {0: quant_strip(0)}
                wq_bufs[1] = quant_strip(1)
                for g in range(ng):
                    for r in range(g * tpg, (g + 1) * tpg):
                        phase_a(r)
                    dq_broadcast(g)
                    for ot in range(not_):
                        gemm_group(wq_bufs[0], 0, ot, g)
                    if g >= 1:
                        for ot in range(not_):
                            gemm_group(wq_bufs[1], 1, ot, g - 1)
                for ot in range(not_):
                    gemm_group(wq_bufs[1], 1, ot, ng - 1)

                # ---- dense strips 2..ns-1 ---------------------------------
                for s in range(2, ns):
                    wq = quant_strip(s)
                    for ot in range(not_):
                        for g in range(ng):
                            gemm_group(wq, s, ot, g)
        if _loop is not None:
            _loop.__exit__(None, None, None)
    return nc


# ---------------------------------------------------------------------------
def shard_inputs(x, norm_weight, weight, n_cores=N_CORES, use_collective=True):
    B, S, D = x.shape
    O = weight.shape[0]
    T_full = B * S
    T = T_full // n_cores

    xf = np.ascontiguousarray(x.reshape(T_full, D), dtype=np.float32)
    wt = np.ascontiguousarray(weight.T.astype(np.float32))
    nww = np.ascontiguousarray(
        np.broadcast_to(norm_weight.astype(np.float32), (128, D))
    )
    d_rows = D // n_cores if use_collective else D
    in_maps = []
    for c in range(n_cores):
        in_maps.append({
            "xin": xf[c * T:(c + 1) * T],
            "wt": wt,
            "wrows": np.ascontiguousarray(wt[c * d_rows:(c + 1) * d_rows])
            if use_collective else wt,
            "nww": nww,
        })
    return in_maps, (B, S, O, T)


def kernel(x, norm_weight, weight):
    """Full-input entry point: shard over 8 cores, run, gather."""
    from concourse.bass_utils import run_bass_kernel_spmd

    in_maps, (B, S, O, T) = shard_inputs(x, norm_weight, weight)
    D = x.shape[2]
    nc = build_bitlinear(T, D, O, n_cores=N_CORES,
                         nw_is_ones=bool(np.all(norm_weight == 1.0)))
    res = run_bass_kernel_spmd(nc, in_maps, list(range(N_CORES)))
    y = np.concatenate(
        [np.ascontiguousarray(res.results[c]["youtT"].T) for c in range(N_CORES)],
        axis=0,
    )
    return np.ascontiguousarray(y.reshape(B, S, O).astype(np.float32))
